# revision 22
# baseline (speedup 1.0000x reference)
"""Trainium2 Bass kernel for 4-D valid convolution (Winograd F(4,3) in z).

Problem: inputs [2, 64, 18, 18, 18, 18] fp32, kernel [81, 64, 64] fp32
(81 = 3^4 offsets row-major over (dw, dx, dy, dz)), output
[2, 64, 16, 16, 16, 16] fp32.

Sharding (8 cores): batch (2) x output-W chunks (4 chunks of 4).  Each core
gets input slabs x[b, :, w0:w0+6] plus the full kernel, and produces
out[b, :, w0:w0+4] as [64, 4, 16, 16, 16] (fp16 on the wire, cast to
fp32 on the host).

The PE is column-issue limited: every matmul step costs ~N cycles at
2.4 GHz regardless of K, with two quadrant streams running concurrently
(col-groups 0/64).  Runtime therefore scales with (#steps x N) =
m-columns per output.  Winograd F(4,3) along z (4 outputs per 6-point
tile, 6 m-phases) cuts m-columns/output to 1.5 (vs 2 for F(2,3), 4 for
direct), and the epilogue applies the 6->4 inverse transform
(out0=M0+S+S2, out1=D+2D2, out2=S+4S2, out3=D+8D2+M5 with S/D =
M1+-M2, S2/D2 = M3+-M4) on ACT+DVE, scheduled incrementally between
phase streams (phase order 1,2,3,4,0,5) so only out3+=M5 and the
stores trail the last matmul.

Layout per (phase k, slab): m_k[ci, X(18), Y(18), zt(4)], col =
X*72 + Y*4 + zt.  The host ships the 128-row dup tile (rows 64-127 =
m_k, rows 0-63 = m_k shifted +4 cols = one y-row); a K=128 matmul at
column q then covers (dy, dy+1) pairs; dy=2 singles run K=64 4-way
packed (tile A on quadrant (0,0) reading lo rows, tile B on (64,64)
reading hi rows, streaming concurrently).

Tiles: 4 tile-pairs (one per w), col-group A = x-planes 0-7, B = 8-15,
N=512 streams (8x*16y*4zt) so the two LDWEIGHTS per step (~54ns each)
hide fully under the 213ns step.  Each phase accumulates into its own
1-bank PSUM tile from an 8-deep pool; the incremental epilogue frees
phase banks early, so the rotation never stalls.  Loads are split
lo-rows/sync, hi-rows/gpsimd, one 2-D descriptor per (phase, slab),
need-ordered (slabs 0-2 of every phase first); stores ride the sync
ring only (SWDGE stores would expose the gpsimd teardown drain).
"""

import os
import sys

import numpy as np

if "/opt/trn_rl_repo" not in sys.path:
    sys.path.insert(0, "/opt/trn_rl_repo")
os.environ.setdefault("JAX_PLATFORMS", "axon,cpu")

B, CIN, COUT = 2, 64, 64
S = 18          # input spatial per dim
SO = 16         # output spatial per dim
NW = 4          # output w per core
NSLAB = 6       # input w slabs per core
NPH = 6         # Winograd F(4,3) m-phases
ZT = 4          # z-tiles (of 4) per output
XPL = S * ZT              # 72 cols per x-plane
DC = S * XPL              # 1296 m-cols per slab
DKC = DC + 4              # cols incl the +4 (one y-row) dup shift
DSL = DKC + 16            # sbuf cols incl rearrange-view slack
PORD = [1, 2, 3, 4, 0, 5]  # phase stream order (see epilogue)

_CACHE = {}

# F(4,3) transform matrices, interpolation points [0, 1, -1, 2, -2, inf]
_AT = np.array(
    [[1, 1, 1, 1, 1, 0],
     [0, 1, -1, 2, -2, 0],
     [0, 1, 1, 4, 4, 0],
     [0, 1, -1, 8, -8, 1]], dtype=np.float64)
_G = np.array(
    [[1 / 4, 0, 0],
     [-1 / 6, -1 / 6, -1 / 6],
     [-1 / 6, 1 / 6, -1 / 6],
     [1 / 24, 1 / 12, 1 / 6],
     [1 / 24, -1 / 12, 1 / 6],
     [0, 0, 1]], dtype=np.float64)
_BT = np.array(
    [[4, 0, -5, 0, 1, 0],
     [0, -4, -4, 1, 1, 0],
     [0, 4, -4, -1, 1, 0],
     [0, -2, -1, 2, 1, 0],
     [0, 2, -1, -2, 1, 0],
     [0, 4, 0, -5, 0, 1]], dtype=np.float64)


def _build_nc(dt_in):
    import concourse.bass as bass
    import concourse.mybir as mybir

    f32 = mybir.dt.float32

    nc = bass.Bass()
    d_h = []
    for k in range(NPH):
        d_t = nc.dram_tensor(
            f"d{k}", [128, NSLAB, DKC], dt_in, kind="ExternalInput"
        )
        d_h.append(d_t)
    # wp{k}: pair weights, lo rows = Gw_k[(dw,dx), dy=0], hi rows = dy=1
    # ws{k}: single weights, both halves = Gw_k[(dw,dx), dy=2]
    wp_h, ws_h = [], []
    for k in range(NPH):
        wp_t = nc.dram_tensor(
            f"wp{k}", [128, 9, COUT], dt_in, kind="ExternalInput"
        )
        ws_t = nc.dram_tensor(
            f"ws{k}", [128, 9, COUT], dt_in, kind="ExternalInput"
        )
        wp_h.append(wp_t)
        ws_h.append(ws_t)
    out_h = nc.dram_tensor(
        "out", [COUT, NW, SO, SO, SO], dt_in, kind="ExternalOutput"
    )

    tc = _make_tile_context(nc)
    with tc:
        with (
            tc.tile_pool(name="xp", bufs=1) as xpool,
            tc.tile_pool(name="wpl", bufs=1) as wpool,
            tc.tile_pool(name="ob", bufs=3) as opool,
            tc.tile_pool(name="sc", bufs=2) as spool,
            tc.tile_pool(name="ps", bufs=8, space="PSUM") as ppool,
        ):
            wps, wss = [], []
            for k in range(NPH):
                wp_t = wpool.tile([128, 9, COUT], dt_in, tag=f"wp{k}")
                ws_t = wpool.tile([128, 9, COUT], dt_in, tag=f"ws{k}")
                wps.append(wp_t)
                wss.append(ws_t)
            ds = []
            for k in range(NPH):
                row = []
                for s in range(NSLAB):
                    d_t = xpool.tile([128, DSL], dt_in, tag=f"d{k}s{s}")
                    row.append(d_t)
                ds.append(row)

            def load_slab(k, s):
                # lo rows on the sync (HWDGE) ring, hi rows on the
                # gpsimd (SWDGE) ring - one 2-D descriptor each
                nc.sync.dma_start(ds[k][s][0:64, 0:DKC], d_h[k][0:64, s, :])
                nc.gpsimd.dma_start(
                    ds[k][s][64:128, 0:DKC], d_h[k][64:128, s, :]
                )

            # need order: first phase's weights lead the sync ring, the
            # other phases' weights ride gpsimd between slab loads; every
            # phase's slabs 0-2 first (tile-pair 1), then slabs 3-5.
            nc.sync.dma_start(wps[PORD[0]][:], wp_h[PORD[0]][:])
            nc.sync.dma_start(wss[PORD[0]][:], ws_h[PORD[0]][:])
            dummy = xpool.tile([64, 6, 64], dt_in, tag="dummy")
            for k in PORD:
                if k != PORD[0]:
                    nc.gpsimd.dma_start(wps[k][:], wp_h[k][:])
                    nc.gpsimd.dma_start(wss[k][:], ws_h[k][:])
                for s in range(3):
                    load_slab(k, s)
                if k == PORD[0]:
                    # SWDGE lane sems lag the ring by several descriptor
                    # positions; these tiny transfers burn positions fast
                    # so the phase-1 hi-slab waits release ~2-3us sooner.
                    for j in range(6):
                        nc.gpsimd.dma_start(
                            dummy[:, j, :], d_h[k][64:128, 0, 0:64]
                        )
            for k in PORD:
                for s in range(3, NSLAB):
                    load_slab(k, s)

            # HAM warmup: the PE clock-gate runs cold (1.2 GHz) until
            # ~3.4us of sustained matmul activity.  Dependency-free
            # matmuls on never-written scratch warm it up while the
            # engine preamble + first loads land (~10us), so the real
            # stream starts near 2.4 GHz.
            warm_ps = ppool.tile([128, 512], f32, tag="ps")
            wscr = xpool.tile([128, 640], dt_in, tag="wscr")
            nc.vector.memset(wscr[:], 0.5)
            for _ in range(7):
                nc.tensor.matmul(
                    warm_ps[0:64, :],
                    wscr[:, 0:64],
                    wscr[:, 64:576],
                    start=True, stop=True,
                    tile_position=(0, 0),
                )

            def rhs(t, prange, q0):
                v = t[prange, q0 : q0 + 8 * XPL]
                v = v.rearrange("p (x y z) -> p x y z", x=8, y=S, z=ZT)
                return v[:, :, 0:16, :]

            PFULL = slice(0, 128)
            PLO = slice(0, 64)
            PHI = slice(64, 128)

            def emit_phase(pk, k, w):
                # dy (0,1) pairs: 9 K=128 matmuls per tile; col-group A =
                # x-planes 0-7, col-group B = 8-15
                for j2 in range(9):
                    dw, dx = j2 // 3, j2 % 3
                    dt_ = ds[k][w + dw]
                    q = dx * XPL + 4
                    st = j2 == 0
                    nc.tensor.matmul(
                        pk[0:64, :],
                        wps[k][:, j2, :],
                        rhs(dt_, PFULL, q),
                        start=st, stop=False,
                        tile_position=(0, 0),
                    )
                    nc.tensor.matmul(
                        pk[64:128, :],
                        wps[k][:, j2, :],
                        rhs(dt_, PFULL, q + 8 * XPL),
                        start=st, stop=False,
                        tile_position=(0, 64),
                    )
                # dy=2 singles: K=64, 4-way packed (A lo / B hi)
                for j2 in range(9):
                    dw, dx = j2 // 3, j2 % 3
                    dt_ = ds[k][w + dw]
                    last = j2 == 8
                    nc.tensor.matmul(
                        pk[0:64, :],
                        wss[k][0:64, j2, :],
                        rhs(dt_, PLO, dx * XPL + 12),
                        start=False, stop=last,
                        tile_position=(0, 0),
                    )
                    nc.tensor.matmul(
                        pk[64:128, :],
                        wss[k][64:128, j2, :],
                        rhs(dt_, PHI, (8 + dx) * XPL + 8),
                        start=False, stop=last,
                        tile_position=(64, 64),
                    )

            AOP = mybir.AluOpType

            def epi_sd(pt, sc):
                # after phases 1,2: S = M1+M2, D = M1-M2
                Ssum, D = sc[:, 0, :], sc[:, 1, :]
                nc.scalar.copy(Ssum, pt[1][:])
                nc.vector.tensor_add(out=Ssum, in0=pt[2][:], in1=Ssum)
                nc.scalar.copy(D, pt[1][:])
                nc.vector.scalar_tensor_tensor(
                    out=D, in0=pt[2][:], scalar=-1.0, in1=D,
                    op0=AOP.mult, op1=AOP.add,
                )

            def epi_s2d2(pt, osb, sc):
                # after phases 3,4: S2/D2 = M3+-M4, then o1, o2, o3-partial
                Ssum, D, S2, D2 = (sc[:, i, :] for i in range(4))
                nc.scalar.copy(S2, pt[3][:])
                nc.vector.tensor_add(out=S2, in0=pt[4][:], in1=S2)
                nc.scalar.copy(D2, pt[3][:])
                nc.vector.scalar_tensor_tensor(
                    out=D2, in0=pt[4][:], scalar=-1.0, in1=D2,
                    op0=AOP.mult, op1=AOP.add,
                )
                nc.vector.scalar_tensor_tensor(
                    out=osb[:, :, 1], in0=D2, scalar=2.0, in1=D,
                    op0=AOP.mult, op1=AOP.add,
                )
                nc.vector.scalar_tensor_tensor(
                    out=osb[:, :, 2], in0=S2, scalar=4.0, in1=Ssum,
                    op0=AOP.mult, op1=AOP.add,
                )
                # o3-partial = D + 8*D2 kept in sc slot 1 (D dies here)
                # so the strided fp16 osb write happens exactly once.
                nc.vector.scalar_tensor_tensor(
                    out=D, in0=D2, scalar=8.0, in1=D,
                    op0=AOP.mult, op1=AOP.add,
                )

            def epi_o0(pt, osb, sc):
                # after phase 0: out0 = M0 + S + S2 (tmp in sc slot 0; S
                # dies here) - single fp16 osb write
                t = sc[:, 0, :]
                nc.vector.tensor_add(out=t, in0=sc[:, 2, :], in1=t)
                nc.vector.tensor_add(out=osb[:, :, 0], in0=pt[0][:], in1=t)

            def epi_o3(pt, osb, sc, hs=slice(0, 512)):
                # after phase 5: out3 = (D + 8*D2) + M5
                nc.vector.tensor_add(
                    out=osb[:, hs, 3], in0=pt[5][:, hs], in1=sc[:, 1, hs]
                )

            def rhs4(t, prange, q0):
                v = t[prange, q0 : q0 + 4 * XPL]
                v = v.rearrange("p (x y z) -> p x y z", x=4, y=S, z=ZT)
                return v[:, :, 0:16, :]

            def emit_phase_half(pk, k, w, h):
                # phase k over psum col-half h (x-planes 4h..4h+3 per
                # col-group), N=256: used for the last tile-pair's final
                # phase so its epilogue+stores pipeline under the stream
                x0 = 4 * h
                cs = slice(256 * h, 256 * h + 256)
                for j2 in range(9):
                    dw, dx = j2 // 3, j2 % 3
                    dt_ = ds[k][w + dw]
                    q = (x0 + dx) * XPL + 4
                    st = j2 == 0
                    nc.tensor.matmul(
                        pk[0:64, cs],
                        wps[k][:, j2, :],
                        rhs4(dt_, PFULL, q),
                        start=st, stop=False,
                        tile_position=(0, 0),
                    )
                    nc.tensor.matmul(
                        pk[64:128, cs],
                        wps[k][:, j2, :],
                        rhs4(dt_, PFULL, q + 8 * XPL),
                        start=st, stop=False,
                        tile_position=(0, 64),
                    )
                for j2 in range(9):
                    dw, dx = j2 // 3, j2 % 3
                    dt_ = ds[k][w + dw]
                    last = j2 == 8
                    nc.tensor.matmul(
                        pk[0:64, cs],
                        wss[k][0:64, j2, :],
                        rhs4(dt_, PLO, (x0 + dx) * XPL + 12),
                        start=False, stop=last,
                        tile_position=(0, 0),
                    )
                    nc.tensor.matmul(
                        pk[64:128, cs],
                        wss[k][64:128, j2, :],
                        rhs4(dt_, PHI, (8 + x0 + dx) * XPL + 8),
                        start=False, stop=last,
                        tile_position=(64, 64),
                    )

            def emit_stores(osb, w, halves=(0, 1)):
                lo = osb[0:64].rearrange(
                    "p (x y zt) r -> p x y (zt r)", x=8, y=16, zt=ZT
                )
                hi = osb[64:128].rearrange(
                    "p (x y zt) r -> p x y (zt r)", x=8, y=16, zt=ZT
                )
                # stores on the HWDGE (sync) ring only: SWDGE stores
                # would hold up the gpsimd teardown drain ~2us.
                for h in halves:
                    c0 = 4 * h
                    nc.sync.dma_start(
                        out_h[:, w, c0 : c0 + 4, :, :], lo[:, c0 : c0 + 4]
                    )
                    nc.sync.dma_start(
                        out_h[:, w, 8 + c0 : 12 + c0, :, :],
                        hi[:, c0 : c0 + 4],
                    )

            # ---- main loop: 4 tile-pairs (one per w) ----
            for w in range(NW):
                last_w = w == NW - 1
                pt = {}
                osb = opool.tile([128, 512, 4], dt_in, tag="osb")
                sc = spool.tile([128, 4, 512], f32, tag="sc")
                for k in PORD:
                    p_t = ppool.tile([128, 512], f32, tag="ps")
                    pt[k] = p_t
                    if k == 5 and last_w:
                        # final phase of the final tile-pair runs per
                        # column-half so epilogue+stores of half 0 hide
                        # under half 1's matmul stream
                        for h in range(2):
                            emit_phase_half(p_t, k, w, h)
                            hs = slice(256 * h, 256 * h + 256)
                            epi_o3(pt, osb, sc, hs)
                            emit_stores(osb, w, halves=(h,))
                        continue
                    emit_phase(p_t, k, w)
                    if k == 2:
                        epi_sd(pt, sc)
                    elif k == 4:
                        epi_s2d2(pt, osb, sc)
                    elif k == 0:
                        epi_o0(pt, osb, sc)
                    elif k == 5:
                        epi_o3(pt, osb, sc)
                if not last_w:
                    emit_stores(osb, w)

    _split_multiwaits(nc)
    return nc


def _make_tile_context(nc):
    from concourse.tile import TileContext

    class TC(TileContext):
        # stock teardown is drain -> barrier -> sem-clear -> barrier; the
        # final barrier only orders engine-stream ends and costs ~2us.
        def _drain_and_barrier(self, tick_clock, wait_clock):
            from concourse.vector_clock import ScopedClock

            nc = self.nc
            drain_inst = nc.sync.drain()
            wait_clock.add_sem_waits(
                drain_inst.ins, ScopedClock({None: tick_clock.global_clock})
            )
            # mark for _split_multiwaits: distribute this drain's waits
            # round-robin across engines (parallel NoOps) instead of ~60
            # serial NoOps on sync (~1.5us tail).  The barrier right
            # after orders every NoOp before the sem clear.  gpsimd is
            # excluded: wait-NoOps there would delay its SWDGE drain
            # (~2.7us of queue teardown) past the barrier, exposing it.
            nc._final_drain_name = drain_inst.ins.name
            nc.all_engine_barrier()
            assert self.sems is not None
            popped = nc._tile_sem_poison_stack.pop()
            assert popped is self._sem_poison
            nc.clear_and_free_semaphores(list(self.sems.allocated().values()))

    return TC(nc)


def _split_multiwaits(nc, max_waits=1):
    """The walrus build here rejects any instruction carrying more than one
    sync-wait ("Too many sync wait commands").  Tile attaches one wait per
    outstanding producer.  Move excess waits onto NoOps inserted
    immediately before the instruction - same-engine, except for the
    teardown drain whose waits are spread round-robin across engines."""
    import concourse.mybir as mybir

    final_drain = getattr(nc, "_final_drain_name", None)
    # EngineType.Pool is the gpsimd queue - excluded (see teardown note)
    spread_engines = [
        e for e in nc.engines if e != mybir.EngineType.Pool
    ] or list(nc.engines)

    n_split = 0
    for fn in nc.m.functions:
        for blk in fn.blocks:
            out = []
            for inst in list(blk.instructions):
                si = inst.sync_info
                if si is not None and si.on_wait and len(si.on_wait) > max_waits:
                    waits = list(si.on_wait)
                    extra = waits[:-max_waits]
                    spread = inst.name == final_drain
                    for k in range(0, len(extra), max_waits):
                        nop = mybir.InstNoOp(
                            name=f"{inst.name}.w{k}", ins=[], outs=[]
                        )
                        if spread:
                            nop.engine = spread_engines[
                                (k // max_waits) % len(spread_engines)
                            ]
                        else:
                            nop.engine = inst.engine
                        nop.sync_info = mybir.SyncInfo(
                            on_wait=extra[k : k + max_waits], on_update=[]
                        )
                        nc.register_instruction(nop)
                        out.append(nop)
                        n_split += 1
                    si.on_wait = waits[-max_waits:]
                out.append(inst)
            blk.instructions = out
    return n_split


# compute dtype: "float16" (fastest, rel err ~1e-3) or "float32r"
DTYPE = "float16"


def _get_nc():
    if "nc" not in _CACHE:
        import concourse.mybir as mybir

        _CACHE["nc"] = _build_nc(getattr(mybir.dt, DTYPE))
    return _CACHE["nc"]


def _np_dtype():
    if DTYPE == "float16":
        return np.float16
    return np.float32


def _shard_inputs(inputs):
    nd = _np_dtype()
    x = np.asarray(inputs["inputs"], dtype=np.float32)
    wk = np.asarray(inputs["kernel"], dtype=np.float32)
    k5 = wk.reshape(3, 3, 3, 3, CIN, COUT)  # [dw, dx, dy, dz, ci, co]
    # weight transform Gw_k over dz
    gw = np.einsum("ij,wxyjcd->iwxycd", _G, k5.astype(np.float64))
    wps, wss = [], []
    for k in range(NPH):
        g = gw[k].reshape(9, 3, CIN, COUT)  # [(dw,dx), dy, ci, co]
        wp = np.concatenate(
            [g[:, 0].transpose(1, 0, 2), g[:, 1].transpose(1, 0, 2)], axis=0
        )
        w2h = g[:, 2].transpose(1, 0, 2)
        ws_ = np.concatenate([w2h, w2h], axis=0)
        wps.append(np.ascontiguousarray(wp.astype(nd)))
        wss.append(np.ascontiguousarray(ws_.astype(nd)))
    in_maps = []
    for c in range(8):
        b, wc = c // 4, c % 4
        w0c = 4 * wc
        sl = x[b, :, w0c : w0c + 6]             # [CIN, 6, 18, 18, 18] fp32
        # z windows of 6, stride 4: zt = 0..3
        xw = np.stack(
            [sl[..., 4 * t : 4 * t + 6] for t in range(ZT)], axis=-2
        )                                        # [CIN, 6, 18, 18, zt, j]
        m = np.einsum("ij,cswxtj->icswxt", _BT, xw)  # [6, CIN, 6, 18, 18, zt]
        feeds = {}
        for k in range(NPH):
            mk = m[k].reshape(CIN, NSLAB, DC).astype(nd)
            dk = np.zeros((128, NSLAB, DKC), dtype=nd)
            dk[0:CIN, :, 4 : DC + 4] = mk       # lo rows: m_k shifted +4
            dk[CIN:, :, 0:DC] = mk              # hi rows: m_k
            feeds[f"d{k}"] = dk
            feeds[f"wp{k}"] = wps[k]
            feeds[f"ws{k}"] = wss[k]
        in_maps.append(feeds)
    return in_maps


def _gather_outputs(results):
    out = np.empty((B, COUT, NW * 4, SO, SO, SO), dtype=np.float32)
    for c in range(8):
        b, wc = c // 4, c % 4
        w0 = 4 * wc
        out[b, :, w0 : w0 + 4] = results[c]["out"].astype(np.float32)
    return out


def kernel(**inputs):
    from concourse.bass_utils import run_bass_kernel_spmd

    res = run_bass_kernel_spmd(_get_nc(), _shard_inputs(inputs), list(range(8)))
    return _gather_outputs(res.results)


# revision 23
# speedup vs baseline: 1.2224x; 1.2224x over previous
"""Trainium2 Bass kernel for 4-D valid convolution (Winograd F(4,3) in z).

Problem: inputs [2, 64, 18, 18, 18, 18] fp32, kernel [81, 64, 64] fp32
(81 = 3^4 offsets row-major over (dw, dx, dy, dz)), output
[2, 64, 16, 16, 16, 16] fp32.

Sharding (8 cores): batch (2) x output-W chunks (4 chunks of 4).  Each core
gets input slabs x[b, :, w0:w0+6] plus the full kernel, and produces
out[b, :, w0:w0+4] as [64, 4, 16, 16, 16] (fp16 on the wire, cast to
fp32 on the host).

The PE is column-issue limited: every matmul step costs ~N cycles at
2.4 GHz regardless of K, with two quadrant streams running concurrently
(col-groups 0/64).  Runtime therefore scales with (#steps x N) =
m-columns per output.  Winograd F(4,3) along z (4 outputs per 6-point
tile, 6 m-phases) cuts m-columns/output to 1.5 (vs 2 for F(2,3), 4 for
direct), and the epilogue applies the 6->4 inverse transform
(out0=M0+S+S2, out1=D+2D2, out2=S+4S2, out3=D+8D2+M5 with S/D =
M1+-M2, S2/D2 = M3+-M4) on ACT+DVE, scheduled incrementally between
phase streams (phase order 1,2,3,4,0,5) so only out3+=M5 and the
stores trail the last matmul.

Layout per (phase k, slab): m_k[ci, X(18), Y(18), zt(4)], col =
X*72 + Y*4 + zt.  The host ships the 128-row dup tile (rows 64-127 =
m_k, rows 0-63 = m_k shifted +4 cols = one y-row); a K=128 matmul at
column q then covers (dy, dy+1) pairs; dy=2 singles run K=64 4-way
packed (tile A on quadrant (0,0) reading lo rows, tile B on (64,64)
reading hi rows, streaming concurrently).

Tiles: 4 tile-pairs (one per w), col-group A = x-planes 0-7, B = 8-15,
N=512 streams (8x*16y*4zt) so the two LDWEIGHTS per step (~54ns each)
hide fully under the 213ns step.  Each phase accumulates into its own
1-bank PSUM tile from an 8-deep pool; the incremental epilogue frees
phase banks early, so the rotation never stalls.  Loads are split
lo-rows/sync, hi-rows/gpsimd, one 2-D descriptor per (phase, slab),
need-ordered (slabs 0-2 of every phase first); stores ride the sync
ring only (SWDGE stores would expose the gpsimd teardown drain).
"""

import os
import sys

import numpy as np

if "/opt/trn_rl_repo" not in sys.path:
    sys.path.insert(0, "/opt/trn_rl_repo")
os.environ.setdefault("JAX_PLATFORMS", "axon,cpu")

B, CIN, COUT = 2, 64, 64
S = 18          # input spatial per dim
SO = 16         # output spatial per dim
NW = 4          # output w per core
NSLAB = 6       # input w slabs per core
NPH = 6         # Winograd F(4,3) m-phases
ZT = 4          # z-tiles (of 4) per output
XPL = S * ZT              # 72 cols per x-plane
DC = S * XPL              # 1296 m-cols per slab
DKC = DC + 4              # cols incl the +4 (one y-row) dup shift
DSL = DKC + 16            # sbuf cols incl rearrange-view slack
PORD = [1, 2, 3, 4, 0, 5]  # phase stream order (see epilogue)

_CACHE = {}

# F(4,3) transform matrices, interpolation points [0, 1, -1, 2, -2, inf]
_AT = np.array(
    [[1, 1, 1, 1, 1, 0],
     [0, 1, -1, 2, -2, 0],
     [0, 1, 1, 4, 4, 0],
     [0, 1, -1, 8, -8, 1]], dtype=np.float64)
_G = np.array(
    [[1 / 4, 0, 0],
     [-1 / 6, -1 / 6, -1 / 6],
     [-1 / 6, 1 / 6, -1 / 6],
     [1 / 24, 1 / 12, 1 / 6],
     [1 / 24, -1 / 12, 1 / 6],
     [0, 0, 1]], dtype=np.float64)
_BT = np.array(
    [[4, 0, -5, 0, 1, 0],
     [0, -4, -4, 1, 1, 0],
     [0, 4, -4, -1, 1, 0],
     [0, -2, -1, 2, 1, 0],
     [0, 2, -1, -2, 1, 0],
     [0, 4, 0, -5, 0, 1]], dtype=np.float64)


def _build_nc(dt_in):
    import concourse.bass as bass
    import concourse.mybir as mybir

    f32 = mybir.dt.float32

    nc = bass.Bass()
    d_h = []
    for k in range(NPH):
        d_t = nc.dram_tensor(
            f"d{k}", [128, NSLAB, DKC], dt_in, kind="ExternalInput"
        )
        d_h.append(d_t)
    # wp{k}: pair weights, lo rows = Gw_k[(dw,dx), dy=0], hi rows = dy=1
    # ws{k}: single weights, both halves = Gw_k[(dw,dx), dy=2]
    wp_h, ws_h = [], []
    for k in range(NPH):
        wp_t = nc.dram_tensor(
            f"wp{k}", [128, 9, COUT], dt_in, kind="ExternalInput"
        )
        ws_t = nc.dram_tensor(
            f"ws{k}", [128, 9, COUT], dt_in, kind="ExternalInput"
        )
        wp_h.append(wp_t)
        ws_h.append(ws_t)
    out_h = nc.dram_tensor(
        "out", [COUT, NW, SO, SO, SO], dt_in, kind="ExternalOutput"
    )

    tc = _make_tile_context(nc)
    with tc:
        with (
            tc.tile_pool(name="xp", bufs=1) as xpool,
            tc.tile_pool(name="wpl", bufs=1) as wpool,
            tc.tile_pool(name="ob", bufs=3) as opool,
            tc.tile_pool(name="sc", bufs=2) as spool,
            tc.tile_pool(name="ps", bufs=8, space="PSUM") as ppool,
        ):
            wps, wss = [], []
            for k in range(NPH):
                wp_t = wpool.tile([128, 9, COUT], dt_in, tag=f"wp{k}")
                ws_t = wpool.tile([128, 9, COUT], dt_in, tag=f"ws{k}")
                wps.append(wp_t)
                wss.append(ws_t)
            ds = []
            for k in range(NPH):
                row = []
                for s in range(NSLAB):
                    d_t = xpool.tile([128, DSL], dt_in, tag=f"d{k}s{s}")
                    row.append(d_t)
                ds.append(row)

            def load_slab(k, s):
                # lo rows on the sync (HWDGE) ring, hi rows on the
                # gpsimd (SWDGE) ring - one 2-D descriptor each
                nc.sync.dma_start(ds[k][s][0:64, 0:DKC], d_h[k][0:64, s, :])
                nc.gpsimd.dma_start(
                    ds[k][s][64:128, 0:DKC], d_h[k][64:128, s, :]
                )

            # need order: first phase's weights lead the sync ring, the
            # other phases' weights ride gpsimd between slab loads; every
            # phase's slabs 0-2 first (tile-pair 1), then slabs 3-5.
            nc.sync.dma_start(wps[PORD[0]][:], wp_h[PORD[0]][:])
            nc.sync.dma_start(wss[PORD[0]][:], ws_h[PORD[0]][:])
            for k in PORD:
                if k != PORD[0]:
                    nc.gpsimd.dma_start(wps[k][:], wp_h[k][:])
                    nc.gpsimd.dma_start(wss[k][:], ws_h[k][:])
                for s in range(3):
                    load_slab(k, s)
            for k in PORD:
                for s in range(3, NSLAB):
                    load_slab(k, s)

            # HAM warmup: the PE clock-gate runs cold (1.2 GHz) until
            # ~3.4us of sustained matmul activity.  Dependency-free
            # matmuls on never-written scratch warm it up while the
            # engine preamble + first loads land (~10us), so the real
            # stream starts near 2.4 GHz.
            warm_ps = ppool.tile([128, 512], f32, tag="ps")
            wscr = xpool.tile([128, 640], dt_in, tag="wscr")
            nc.vector.memset(wscr[:], 0.5)
            for _ in range(7):
                nc.tensor.matmul(
                    warm_ps[0:64, :],
                    wscr[:, 0:64],
                    wscr[:, 64:576],
                    start=True, stop=True,
                    tile_position=(0, 0),
                )

            def rhs(t, prange, q0):
                v = t[prange, q0 : q0 + 8 * XPL]
                v = v.rearrange("p (x y z) -> p x y z", x=8, y=S, z=ZT)
                return v[:, :, 0:16, :]

            PFULL = slice(0, 128)
            PLO = slice(0, 64)
            PHI = slice(64, 128)

            def emit_phase(pk, k, w):
                # dy (0,1) pairs: 9 K=128 matmuls per tile; col-group A =
                # x-planes 0-7, col-group B = 8-15
                for j2 in range(9):
                    dw, dx = j2 // 3, j2 % 3
                    dt_ = ds[k][w + dw]
                    q = dx * XPL + 4
                    st = j2 == 0
                    nc.tensor.matmul(
                        pk[0:64, :],
                        wps[k][:, j2, :],
                        rhs(dt_, PFULL, q),
                        start=st, stop=False,
                        tile_position=(0, 0),
                    )
                    nc.tensor.matmul(
                        pk[64:128, :],
                        wps[k][:, j2, :],
                        rhs(dt_, PFULL, q + 8 * XPL),
                        start=st, stop=False,
                        tile_position=(0, 64),
                    )
                # dy=2 singles: K=64, 4-way packed (A lo / B hi)
                for j2 in range(9):
                    dw, dx = j2 // 3, j2 % 3
                    dt_ = ds[k][w + dw]
                    last = j2 == 8
                    nc.tensor.matmul(
                        pk[0:64, :],
                        wss[k][0:64, j2, :],
                        rhs(dt_, PLO, dx * XPL + 12),
                        start=False, stop=last,
                        tile_position=(0, 0),
                    )
                    nc.tensor.matmul(
                        pk[64:128, :],
                        wss[k][64:128, j2, :],
                        rhs(dt_, PHI, (8 + dx) * XPL + 8),
                        start=False, stop=last,
                        tile_position=(64, 64),
                    )

            AOP = mybir.AluOpType

            def epi_sd(pt, sc):
                # after phases 1,2: S = M1+M2, D = M1-M2
                Ssum, D = sc[:, 0, :], sc[:, 1, :]
                nc.scalar.copy(Ssum, pt[1][:])
                nc.vector.tensor_add(out=Ssum, in0=pt[2][:], in1=Ssum)
                nc.scalar.copy(D, pt[1][:])
                nc.vector.scalar_tensor_tensor(
                    out=D, in0=pt[2][:], scalar=-1.0, in1=D,
                    op0=AOP.mult, op1=AOP.add,
                )

            def epi_s2d2(pt, osb, sc):
                # after phases 3,4: S2/D2 = M3+-M4, then o1, o2, o3-partial
                Ssum, D, S2, D2 = (sc[:, i, :] for i in range(4))
                nc.scalar.copy(S2, pt[3][:])
                nc.vector.tensor_add(out=S2, in0=pt[4][:], in1=S2)
                nc.scalar.copy(D2, pt[3][:])
                nc.vector.scalar_tensor_tensor(
                    out=D2, in0=pt[4][:], scalar=-1.0, in1=D2,
                    op0=AOP.mult, op1=AOP.add,
                )
                nc.vector.scalar_tensor_tensor(
                    out=osb[:, :, 1], in0=D2, scalar=2.0, in1=D,
                    op0=AOP.mult, op1=AOP.add,
                )
                nc.vector.scalar_tensor_tensor(
                    out=osb[:, :, 2], in0=S2, scalar=4.0, in1=Ssum,
                    op0=AOP.mult, op1=AOP.add,
                )
                # o3-partial = D + 8*D2 kept in sc slot 1 (D dies here)
                # so the strided fp16 osb write happens exactly once.
                nc.vector.scalar_tensor_tensor(
                    out=D, in0=D2, scalar=8.0, in1=D,
                    op0=AOP.mult, op1=AOP.add,
                )

            def epi_o0(pt, osb, sc):
                # after phase 0: out0 = M0 + S + S2 (tmp in sc slot 0; S
                # dies here) - single fp16 osb write
                t = sc[:, 0, :]
                nc.vector.tensor_add(out=t, in0=sc[:, 2, :], in1=t)
                nc.vector.tensor_add(out=osb[:, :, 0], in0=pt[0][:], in1=t)

            def epi_o3(pt, osb, sc, hs=slice(0, 512)):
                # after phase 5: out3 = (D + 8*D2) + M5
                nc.vector.tensor_add(
                    out=osb[:, hs, 3], in0=pt[5][:, hs], in1=sc[:, 1, hs]
                )

            def rhs4(t, prange, q0):
                v = t[prange, q0 : q0 + 4 * XPL]
                v = v.rearrange("p (x y z) -> p x y z", x=4, y=S, z=ZT)
                return v[:, :, 0:16, :]

            def emit_phase_half(pk, k, w, h):
                # phase k over psum col-half h (x-planes 4h..4h+3 per
                # col-group), N=256: used for the last tile-pair's final
                # phase so its epilogue+stores pipeline under the stream
                x0 = 4 * h
                cs = slice(256 * h, 256 * h + 256)
                for j2 in range(9):
                    dw, dx = j2 // 3, j2 % 3
                    dt_ = ds[k][w + dw]
                    q = (x0 + dx) * XPL + 4
                    st = j2 == 0
                    nc.tensor.matmul(
                        pk[0:64, cs],
                        wps[k][:, j2, :],
                        rhs4(dt_, PFULL, q),
                        start=st, stop=False,
                        tile_position=(0, 0),
                    )
                    nc.tensor.matmul(
                        pk[64:128, cs],
                        wps[k][:, j2, :],
                        rhs4(dt_, PFULL, q + 8 * XPL),
                        start=st, stop=False,
                        tile_position=(0, 64),
                    )
                for j2 in range(9):
                    dw, dx = j2 // 3, j2 % 3
                    dt_ = ds[k][w + dw]
                    last = j2 == 8
                    nc.tensor.matmul(
                        pk[0:64, cs],
                        wss[k][0:64, j2, :],
                        rhs4(dt_, PLO, (x0 + dx) * XPL + 12),
                        start=False, stop=last,
                        tile_position=(0, 0),
                    )
                    nc.tensor.matmul(
                        pk[64:128, cs],
                        wss[k][64:128, j2, :],
                        rhs4(dt_, PHI, (8 + x0 + dx) * XPL + 8),
                        start=False, stop=last,
                        tile_position=(64, 64),
                    )

            def emit_stores(osb, w, halves=(0, 1)):
                lo = osb[0:64].rearrange(
                    "p (x y zt) r -> p x y (zt r)", x=8, y=16, zt=ZT
                )
                hi = osb[64:128].rearrange(
                    "p (x y zt) r -> p x y (zt r)", x=8, y=16, zt=ZT
                )
                # stores on the HWDGE (sync) ring only: SWDGE stores
                # would hold up the gpsimd teardown drain ~2us.
                for h in halves:
                    c0 = 4 * h
                    nc.sync.dma_start(
                        out_h[:, w, c0 : c0 + 4, :, :], lo[:, c0 : c0 + 4]
                    )
                    nc.sync.dma_start(
                        out_h[:, w, 8 + c0 : 12 + c0, :, :],
                        hi[:, c0 : c0 + 4],
                    )

            # ---- main loop: 4 tile-pairs (one per w) ----
            for w in range(NW):
                last_w = w == NW - 1
                pt = {}
                osb = opool.tile([128, 512, 4], dt_in, tag="osb")
                sc = spool.tile([128, 4, 512], f32, tag="sc")
                for k in PORD:
                    p_t = ppool.tile([128, 512], f32, tag="ps")
                    pt[k] = p_t
                    if k == 5 and last_w:
                        # final phase of the final tile-pair runs per
                        # column-half so epilogue+stores of half 0 hide
                        # under half 1's matmul stream
                        for h in range(2):
                            emit_phase_half(p_t, k, w, h)
                            hs = slice(256 * h, 256 * h + 256)
                            epi_o3(pt, osb, sc, hs)
                            emit_stores(osb, w, halves=(h,))
                        continue
                    emit_phase(p_t, k, w)
                    if k == 2:
                        epi_sd(pt, sc)
                    elif k == 4:
                        epi_s2d2(pt, osb, sc)
                    elif k == 0:
                        epi_o0(pt, osb, sc)
                    elif k == 5:
                        epi_o3(pt, osb, sc)
                if not last_w:
                    emit_stores(osb, w)

    _split_multiwaits(nc)
    return nc


def _make_tile_context(nc):
    from concourse.tile import TileContext

    class TC(TileContext):
        # stock teardown is drain -> barrier -> sem-clear -> barrier; the
        # final barrier only orders engine-stream ends and costs ~2us.
        def _drain_and_barrier(self, tick_clock, wait_clock):
            from concourse.vector_clock import ScopedClock

            nc = self.nc
            drain_inst = nc.sync.drain()
            wait_clock.add_sem_waits(
                drain_inst.ins, ScopedClock({None: tick_clock.global_clock})
            )
            # mark for _split_multiwaits: distribute this drain's waits
            # round-robin across engines (parallel NoOps) instead of ~60
            # serial NoOps on sync (~1.5us tail).  The barrier right
            # after orders every NoOp before the sem clear.  gpsimd is
            # excluded: wait-NoOps there would delay its SWDGE drain
            # (~2.7us of queue teardown) past the barrier, exposing it.
            nc._final_drain_name = drain_inst.ins.name
            nc.all_engine_barrier()
            assert self.sems is not None
            popped = nc._tile_sem_poison_stack.pop()
            assert popped is self._sem_poison
            nc.clear_and_free_semaphores(list(self.sems.allocated().values()))

    return TC(nc)


def _split_multiwaits(nc, max_waits=1):
    """The walrus build here rejects any instruction carrying more than one
    sync-wait ("Too many sync wait commands").  Tile attaches one wait per
    outstanding producer.  Move excess waits onto NoOps inserted
    immediately before the instruction - same-engine, except for the
    teardown drain whose waits are spread round-robin across engines."""
    import concourse.mybir as mybir

    final_drain = getattr(nc, "_final_drain_name", None)
    # EngineType.Pool is the gpsimd queue - excluded (see teardown note)
    spread_engines = [
        e for e in nc.engines if e != mybir.EngineType.Pool
    ] or list(nc.engines)

    n_split = 0
    for fn in nc.m.functions:
        for blk in fn.blocks:
            out = []
            for inst in list(blk.instructions):
                si = inst.sync_info
                if si is not None and si.on_wait and len(si.on_wait) > max_waits:
                    waits = list(si.on_wait)
                    extra = waits[:-max_waits]
                    spread = inst.name == final_drain
                    for k in range(0, len(extra), max_waits):
                        nop = mybir.InstNoOp(
                            name=f"{inst.name}.w{k}", ins=[], outs=[]
                        )
                        if spread:
                            nop.engine = spread_engines[
                                (k // max_waits) % len(spread_engines)
                            ]
                        else:
                            nop.engine = inst.engine
                        nop.sync_info = mybir.SyncInfo(
                            on_wait=extra[k : k + max_waits], on_update=[]
                        )
                        nc.register_instruction(nop)
                        out.append(nop)
                        n_split += 1
                    si.on_wait = waits[-max_waits:]
                out.append(inst)
            blk.instructions = out
    return n_split


# compute dtype: "float16" (fastest, rel err ~1e-3) or "float32r"
DTYPE = "float16"


def _get_nc():
    if "nc" not in _CACHE:
        import concourse.mybir as mybir

        _CACHE["nc"] = _build_nc(getattr(mybir.dt, DTYPE))
    return _CACHE["nc"]


def _np_dtype():
    if DTYPE == "float16":
        return np.float16
    return np.float32


def _shard_inputs(inputs):
    nd = _np_dtype()
    x = np.asarray(inputs["inputs"], dtype=np.float32)
    wk = np.asarray(inputs["kernel"], dtype=np.float32)
    k5 = wk.reshape(3, 3, 3, 3, CIN, COUT)  # [dw, dx, dy, dz, ci, co]
    # weight transform Gw_k over dz
    gw = np.einsum("ij,wxyjcd->iwxycd", _G, k5.astype(np.float64))
    wps, wss = [], []
    for k in range(NPH):
        g = gw[k].reshape(9, 3, CIN, COUT)  # [(dw,dx), dy, ci, co]
        wp = np.concatenate(
            [g[:, 0].transpose(1, 0, 2), g[:, 1].transpose(1, 0, 2)], axis=0
        )
        w2h = g[:, 2].transpose(1, 0, 2)
        ws_ = np.concatenate([w2h, w2h], axis=0)
        wps.append(np.ascontiguousarray(wp.astype(nd)))
        wss.append(np.ascontiguousarray(ws_.astype(nd)))
    in_maps = []
    for c in range(8):
        b, wc = c // 4, c % 4
        w0c = 4 * wc
        sl = x[b, :, w0c : w0c + 6]             # [CIN, 6, 18, 18, 18] fp32
        # z windows of 6, stride 4: zt = 0..3
        xw = np.stack(
            [sl[..., 4 * t : 4 * t + 6] for t in range(ZT)], axis=-2
        )                                        # [CIN, 6, 18, 18, zt, j]
        m = np.einsum("ij,cswxtj->icswxt", _BT, xw)  # [6, CIN, 6, 18, 18, zt]
        feeds = {}
        for k in range(NPH):
            mk = m[k].reshape(CIN, NSLAB, DC).astype(nd)
            dk = np.zeros((128, NSLAB, DKC), dtype=nd)
            dk[0:CIN, :, 4 : DC + 4] = mk       # lo rows: m_k shifted +4
            dk[CIN:, :, 0:DC] = mk              # hi rows: m_k
            feeds[f"d{k}"] = dk
            feeds[f"wp{k}"] = wps[k]
            feeds[f"ws{k}"] = wss[k]
        in_maps.append(feeds)
    return in_maps


def _gather_outputs(results):
    out = np.empty((B, COUT, NW * 4, SO, SO, SO), dtype=np.float32)
    for c in range(8):
        b, wc = c // 4, c % 4
        w0 = 4 * wc
        out[b, :, w0 : w0 + 4] = results[c]["out"].astype(np.float32)
    return out


def kernel(**inputs):
    from concourse.bass_utils import run_bass_kernel_spmd

    res = run_bass_kernel_spmd(_get_nc(), _shard_inputs(inputs), list(range(8)))
    return _gather_outputs(res.results)
